# revision 2
# baseline (speedup 1.0000x reference)
"""nn_Attention — tensor-parallel causal attention on 8 TRN2 NeuronCores. v2.

Changes vs v1 (334us):
- Denominator: instead of a per-block ones-matmul (29us of PE), the exp
  blocks are accumulated on the DVE (f32 acc, last add rounds to bf16) and
  ONE ones-matmul of N=512 per (head, q-chunk) broadcasts the row sums
  (3.4us of PE). Same numerics class: f32 sums, one bf16 round.
- Phase 2 loops q-chunk-outer / head-inner, with the c_proj rows for each
  chunk interleaved right after it: phase-3 PE work unlocks early so the
  PE stays fed while ScalarE grinds exp.
- PSUM: sc 3 bufs, out 2, den 1, proj 2 (8 banks).
- eb0 wq DMA split so the first matmul gates on 32KB, not 256KB.
"""

import os
import sys

for _p in ("/opt/trn_rl_repo", "/root/.axon_site/_ro/trn_rl_repo"):
    if os.path.isdir(_p) and _p not in sys.path:
        sys.path.append(_p)

from contextlib import ExitStack

import numpy as np

import concourse.bass as bass
import concourse.tile as tile
from concourse import bacc, mybir
from concourse.bass_utils import run_bass_kernel_spmd

F32 = mybir.dt.float32
BF16 = mybir.dt.bfloat16
P = 128
CHUNK = 512
DIAG = CHUNK // P

S, E, NHEAD = 2048, 2048, 16
BATCH = 2
H = 4            # heads per core
NJ = 3 * H       # j-blocks in wqkv slice
NQK = 2 * H      # transposed-projection j-blocks (q,k only)
EB = E // P
SC = S // CHUNK
SB = S // P
EC = E // CHUNK
N_CORES = 8


def _emit(nc):
    xT = nc.dram_tensor("xT", [E, S], BF16, kind="ExternalInput").ap()
    wqkv = nc.dram_tensor("wqkv", [E, NJ * P], BF16, kind="ExternalInput").ap()
    bqkv = nc.dram_tensor("bqkv", [P, NJ], F32, kind="ExternalInput").ap()
    wproj = nc.dram_tensor("wproj", [H * P, E], BF16, kind="ExternalInput").ap()
    tri = nc.dram_tensor("tri", [P, P], BF16, kind="ExternalInput").ap()
    ones = nc.dram_tensor("ones", [P, P], BF16, kind="ExternalInput").ap()
    bv = nc.dram_tensor("bv", [P, H * P], F32, kind="ExternalInput").ap()
    y = nc.dram_tensor("y", [S, E], BF16, kind="ExternalOutput").ap()

    xT_d = xT.rearrange("(eb p) s -> eb p s", p=P)
    wqkv_d = wqkv.rearrange("(eb p) j -> eb p j", p=P)
    wproj_d = wproj.rearrange("(hb p) e -> hb p e", p=P)

    with tile.TileContext(nc) as tc, ExitStack() as ctx:
        const = ctx.enter_context(tc.tile_pool(name="const", bufs=1))
        qkvT_pool = ctx.enter_context(tc.tile_pool(name="qkvT", bufs=1))
        vnat_pool = ctx.enter_context(tc.tile_pool(name="vnat", bufs=1))
        outT_pool = ctx.enter_context(tc.tile_pool(name="outT", bufs=1))
        # PSUM: 3 (scores) + 2 (av out) + 1 (den) + 2 (c_proj) = 8 banks
        psum_sc = ctx.enter_context(tc.tile_pool(name="psum_sc", bufs=3, space="PSUM"))
        psum_out = ctx.enter_context(tc.tile_pool(name="psum_out", bufs=2, space="PSUM"))
        psum_den = ctx.enter_context(tc.tile_pool(name="psum_den", bufs=1, space="PSUM"))
        psum_p3 = ctx.enter_context(tc.tile_pool(name="psum_p3", bufs=2, space="PSUM"))

        bq_t = const.tile([P, NJ], F32)
        tri_t = const.tile([P, P], BF16)
        ones_t = const.tile([P, P], BF16)
        bv_t = const.tile([P, H * P], F32)

        qkT = [qkvT_pool.tile([P, S], BF16, name=f"qkT{jb}") for jb in range(NQK)]
        vnat = [vnat_pool.tile([P, H * P], BF16, name=f"vn{sb}") for sb in range(SB)]
        outT = [outT_pool.tile([P, S], BF16, name=f"outT{h}") for h in range(H)]

        with tc.tile_pool(name="wq", bufs=1) as wq_pool, tc.tile_pool(
            name="xTs", bufs=1
        ) as xT_pool:
            # ---- streamed weight/xT loads, interleaved across queues ----
            wq_tiles = []
            xT_tiles = []
            dma_q = [nc.sync, nc.scalar, nc.gpsimd]
            crit_q = [
                nc.gpsimd if eb % 2 == 1 else (nc.sync if eb % 4 == 0 else nc.scalar)
                for eb in range(EB)
            ]
            for eb in range(EB):
                q = crit_q[eb]
                xt = xT_pool.tile([P, S], BF16, name=f"xT{eb}")
                (nc.scalar if eb == 0 else q).dma_start(
                    xt[:, 0:CHUNK], xT_d[eb][:, 0:CHUNK]
                )
                xT_tiles.append(xt)
                t = wq_pool.tile([P, NJ * P], BF16, name=f"wq{eb}")
                if eb < 4:
                    # jb0 slice alone so the first matmuls gate on 32KB
                    q.dma_start(t[:, 0:P], wqkv_d[eb][:, 0:P])
                    q.dma_start(t[:, P : NQK * P], wqkv_d[eb][:, P : NQK * P])
                else:
                    q.dma_start(t[:, 0 : NQK * P], wqkv_d[eb][:, 0 : NQK * P])
                wq_tiles.append(t)
            for eb in range(EB):
                crit_q[eb].dma_start(
                    wq_tiles[eb][:, NQK * P : NJ * P], wqkv_d[eb][:, NQK * P : NJ * P]
                )
            nc.sync.dma_start(bq_t[:], bqkv[:])
            nc.scalar.dma_start(bv_t[:], bv[:])
            nc.sync.dma_start(tri_t[:], tri[:])
            nc.scalar.dma_start(ones_t[:], ones[:])
            qi = 0
            for sc in range(1, SC):
                s0 = sc * CHUNK
                for eb in range(EB):
                    dma_q[qi % 3].dma_start(
                        xT_tiles[eb][:, s0 : s0 + CHUNK], xT_d[eb][:, s0 : s0 + CHUNK]
                    )
                    qi += 1
            # ---- phase 1: qkT (transposed) + v (natural) ----
            arrival = [0, 1, 2, 3, 5, 4, 6, 7, 9, 8, 10, 11, 13, 12, 14, 15]
            for sc in range(SC):
                s0 = sc * CHUNK
                eb_order = arrival if sc == 0 else list(range(EB))
                ps = [psum_sc.tile([P, CHUNK], F32, name="ps_a") for _ in range(3)]
                ps += [psum_out.tile([P, CHUNK], F32, name="ps_out") for _ in range(2)]
                ps += [psum_den.tile([P, CHUNK], F32, name="ps_den")]
                ps += [psum_p3.tile([P, CHUNK], F32, name="ps_p3") for _ in range(2)]
                for k, eb in enumerate(eb_order):
                    for jb in range(NQK):
                        nc.tensor.matmul(
                            ps[jb][:],
                            wq_tiles[eb][:, jb * P : (jb + 1) * P],
                            xT_tiles[eb][:, s0 : s0 + CHUNK],
                            start=(k == 0),
                            stop=(k == EB - 1),
                        )
                for jb in range(NQK):
                    nc.vector.tensor_scalar_add(
                        qkT[jb][:, s0 : s0 + CHUNK],
                        ps[jb][:],
                        bq_t[:, jb : jb + 1],
                    )
                for r in range(DIAG):
                    sb = sc * DIAG + r
                    ps = psum_sc.tile([P, H * P], F32, name="ps_a")
                    for eb in range(EB):
                        nc.tensor.matmul(
                            ps[:],
                            xT_tiles[eb][:, s0 + r * P : s0 + (r + 1) * P],
                            wq_tiles[eb][:, NQK * P : NJ * P],
                            start=(eb == 0),
                            stop=(eb == EB - 1),
                        )
                    nc.vector.tensor_add(vnat[sb][:], ps[:], bv_t[:])

        # phase-2/3-only pools and wp loads: into the space freed by the
        # phase-1 wq/xT pools
        wp_pool = ctx.enter_context(tc.tile_pool(name="wp", bufs=1))
        exp_pool = ctx.enter_context(tc.tile_pool(name="exp", bufs=10))
        acc_pool = ctx.enter_context(tc.tile_pool(name="acc", bufs=12))
        recip_pool = ctx.enter_context(tc.tile_pool(name="recip", bufs=3))
        unnorm_pool = ctx.enter_context(tc.tile_pool(name="unnorm", bufs=3))
        yout_pool = ctx.enter_context(tc.tile_pool(name="yout", bufs=3))
        wp_tiles = []
        for hb in range(H):
            t = wp_pool.tile([P, E], BF16, name=f"wp{hb}")
            dma_q[hb % 3].dma_start(t[:], wproj_d[hb])
            wp_tiles.append(t)

        # ---- phase 2 + 3, software-pipelined at instruction level ----
        # Engines execute their streams in emission order, so phase-3 matmul
        # groups of chunk ci-1 are emitted as PE filler BETWEEN phase-2
        # blocks of chunk ci (per-block PE work 0.42us < exp 0.65us).
        def p3_group_gen(ci, tail=False):
            """Yield one c_proj (sb, ec) group per next() for chunk ci."""
            for r in range(DIAG):
                sb = ci * DIAG + r
                ot = yout_pool.tile([P, E], BF16, name="yo")
                for ec in range(EC):
                    ps3 = psum_p3.tile([P, CHUNK], F32, name="ps_p3")
                    for h in range(H):
                        nc.tensor.matmul(
                            ps3[:],
                            outT[h][:, sb * P : (sb + 1) * P],
                            wp_tiles[h][:, ec * CHUNK : (ec + 1) * CHUNK],
                            start=(h == 0),
                            stop=(h == H - 1),
                        )
                    # drains: scalar is exp-bound so it takes ~1/4, DVE the
                    # rest; in the tail (after the last exp) scalar is idle
                    if tail or (sb + ec) % 4 == 0:
                        nc.scalar.copy(ot[:, ec * CHUNK : (ec + 1) * CHUNK], ps3[:])
                    else:
                        nc.vector.tensor_copy(
                            ot[:, ec * CHUNK : (ec + 1) * CHUNK], ps3[:]
                        )
                    if sb == SB - 1:
                        (nc.gpsimd if ec % 2 == 0 else nc.sync).dma_start(
                            y[sb * P : (sb + 1) * P, ec * CHUNK : (ec + 1) * CHUNK],
                            ot[:, ec * CHUNK : (ec + 1) * CHUNK],
                        )
                if sb < SB - 1:
                    [nc.sync, nc.gpsimd][sb % 2].dma_start(
                        y[sb * P : (sb + 1) * P, :], ot[:]
                    )
                yield

        filler = None
        deficit = 0.0
        for ci in range(SC):
            i0 = ci * CHUNK
            njb = (ci + 1) * DIAG
            for h in range(H):
                qT, kT = qkT[h], qkT[H + h]
                out_ps = psum_out.tile([P, CHUNK], F32, name="ps_out")
                exs = []
                lvl = []
                for jb in range(njb):
                    dt = jb - DIAG * ci
                    off = max(dt, 0) * P
                    w = CHUNK - off
                    sc_ps = psum_sc.tile([P, CHUNK], F32, name="ps_a")
                    nc.tensor.matmul(
                        sc_ps[:, 0:w],
                        kT[:, jb * P : (jb + 1) * P],
                        qT[:, i0 + off : i0 + CHUNK],
                        start=True,
                        stop=True,
                    )
                    ex = exp_pool.tile([P, CHUNK], BF16, name="ex")
                    nc.scalar.activation(
                        ex[:, 0:w], sc_ps[:, 0:w], mybir.ActivationFunctionType.Exp
                    )
                    if dt >= 0:
                        nc.vector.tensor_mul(ex[:, 0:P], ex[:, 0:P], tri_t[:])
                    nc.tensor.matmul(
                        out_ps[:, off:CHUNK],
                        vnat[jb][:, h * P : (h + 1) * P],
                        ex[:, 0:w],
                        start=(jb == 0),
                        stop=(jb == njb - 1),
                    )
                    exs.append(ex)
                    # denominator level-1 (bf16, on DVE) emitted inline so ex
                    # buffers free promptly (engines run in emission order)
                    if jb < njb - 4 and jb % 2 == 1:
                        t = acc_pool.tile([P, CHUNK], BF16, name="acc")
                        nc.vector.tensor_add(t[:], exs[jb - 1][:], exs[jb][:])
                        lvl.append(t)
                    elif jb == njb - 3:  # after diag blocks dt0, dt1
                        d0, d1 = exs[jb - 1], exs[jb]
                        tA = acc_pool.tile([P, CHUNK], BF16, name="acc")
                        nc.vector.tensor_add(
                            tA[:, P:CHUNK], d0[:, P:CHUNK], d1[:, 0 : CHUNK - P]
                        )
                        nc.vector.tensor_copy(tA[:, 0:P], d0[:, 0:P])
                    elif jb == njb - 1:  # after diag blocks dt2, dt3
                        d2, d3 = exs[jb - 1], exs[jb]
                        tB = acc_pool.tile([P, CHUNK], BF16, name="acc")
                        nc.vector.tensor_add(
                            tB[:, 3 * P : CHUNK], d2[:, P : 2 * P], d3[:, 0:P]
                        )
                        nc.vector.tensor_copy(tB[:, 2 * P : 3 * P], d2[:, 0:P])
                        tD = acc_pool.tile([P, CHUNK], BF16, name="acc")
                        nc.vector.tensor_add(
                            tD[:, 2 * P : CHUNK],
                            tA[:, 2 * P : CHUNK],
                            tB[:, 2 * P : CHUNK],
                        )
                        nc.vector.tensor_copy(tD[:, 0 : 2 * P], tA[:, 0 : 2 * P])
                        lvl.append(tD)
                    # pace phase-3 filler by the exp-vs-PE deficit model
                    deficit += (0.15 + w * 0.00098) - w * 0.000834
                    if filler is not None and deficit >= 0.86:
                        if next(filler, None) is not None or True:
                            deficit -= 0.86
                # upper tree levels
                while len(lvl) > 1:
                    nxt = []
                    for p in range(0, len(lvl) - 1, 2):
                        t = acc_pool.tile([P, CHUNK], BF16, name="acc")
                        nc.vector.tensor_add(t[:], lvl[p][:], lvl[p + 1][:])
                        nxt.append(t)
                    if len(lvl) % 2:
                        nxt.append(lvl[-1])
                    lvl = nxt
                # free out_ps promptly; normalize later from SBUF
                un = unnorm_pool.tile([P, CHUNK], F32, name="un")
                nc.vector.tensor_copy(un[:], out_ps[:])
                den_ps = psum_den.tile([P, CHUNK], F32, name="ps_den")
                nc.tensor.matmul(den_ps[:], ones_t[:], lvl[0][:], start=True, stop=True)
                rc = recip_pool.tile([P, CHUNK], F32, name="rc")
                nc.vector.reciprocal_approx_fast(rc[:], den_ps[:])
                nc.vector.tensor_mul(outT[h][:, i0 : i0 + CHUNK], un[:], rc[:])
            # drain any leftover filler groups of the previous chunk
            if filler is not None:
                for _ in filler:
                    pass
            filler = p3_group_gen(ci, tail=(ci == SC - 1))
        # last chunk's phase 3
        for _ in filler:
            pass
    return nc


_NC = None
LAST_RESULTS = None


def _get_nc():
    global _NC
    if _NC is None:
        nc = bacc.Bacc(
            "TRN2", target_bir_lowering=False, debug=False, num_devices=N_CORES
        )
        _emit(nc)
        nc.compile()
        _NC = nc
    return _NC


def _prep_shared(hidden_states, c_attn_w, c_attn_b, c_proj_w):
    """Host-side prep shared across cores."""
    import ml_dtypes

    bf16 = ml_dtypes.bfloat16
    scale = 1.0 / float(np.sqrt(P))
    xT = [
        np.ascontiguousarray(hidden_states[b].T).astype(bf16) for b in range(BATCH)
    ]
    pp = np.arange(P)
    tri = (pp[:, None] <= pp[None, :]).astype(bf16)
    ones = np.ones((P, P), dtype=bf16)
    return xT, tri, ones, scale, bf16


def _core_inputs(shared, c_attn_w, c_attn_b, c_proj_w, core):
    xT, tri, ones, scale, bf16 = shared
    b, g = core // 4, core % 4
    h0 = H * g
    cols = []
    for part in range(3):
        for h in range(h0, h0 + H):
            base = part * E + h * P
            cols.extend(range(base, base + P))
    cols = np.asarray(cols)
    wqkv = np.ascontiguousarray(c_attn_w[:, cols]).astype(np.float32)
    bq = np.ascontiguousarray(c_attn_b[cols]).astype(np.float32)
    wqkv[:, 0 : H * P] *= scale
    bq[0 : H * P] *= scale
    bq = bq.reshape(NJ, P).T.copy()
    wproj = np.ascontiguousarray(c_proj_w[h0 * P : (h0 + H) * P, :]).astype(bf16)
    bv = np.broadcast_to(bq[:, NQK:NJ].T.reshape(1, H * P), (P, H * P)).copy()
    return {
        "xT": xT[b],
        "wqkv": wqkv.astype(bf16),
        "bqkv": bq,
        "wproj": wproj,
        "tri": tri,
        "ones": ones,
        "bv": bv,
    }


def kernel(hidden_states, c_attn_w, c_attn_b, c_proj_w, c_proj_b):
    global LAST_RESULTS
    hidden_states = np.asarray(hidden_states)
    c_attn_w = np.asarray(c_attn_w)
    c_attn_b = np.asarray(c_attn_b)
    c_proj_w = np.asarray(c_proj_w)
    c_proj_b = np.asarray(c_proj_b)

    nc = _get_nc()
    shared = _prep_shared(hidden_states, c_attn_w, c_attn_b, c_proj_w)
    in_maps = [
        _core_inputs(shared, c_attn_w, c_attn_b, c_proj_w, c)
        for c in range(N_CORES)
    ]
    res = run_bass_kernel_spmd(nc, in_maps, list(range(N_CORES)))
    LAST_RESULTS = res
    out = np.zeros((BATCH, S, E), dtype=np.float32)
    for c in range(N_CORES):
        out[c // 4] += res.results[c]["y"].astype(np.float32)
    out += c_proj_b.astype(np.float32)[None, None, :]
    return out


# revision 3
# speedup vs baseline: 1.0200x; 1.0200x over previous
"""nn_Attention — tensor-parallel causal attention on 8 TRN2 NeuronCores. v8.

vs v7 (333us): phase-1 chunks sc>=1 run the qk projection jb-major (one
PSUM bank per j-block accumulated over all 16 ebs) instead of eb-major,
freeing 6 PSUM banks. The phase-2 attention units of chunk ci=sc-1 are
emitted interleaved into the phase-1 streams (engines execute in emission
order): ScalarE exp for chunks 0-2 is prepaid inside the phase-1 window
where ScalarE idles, so the post-phase-1 stretch (chunk-3 attention +
all c_proj) is PE-bound instead of exp-bound. Denominator row-sums stay
on the DVE bf16 tree + one N=512 ones-matmul per (head, chunk).
"""

import os
import sys

for _p in ("/opt/trn_rl_repo", "/root/.axon_site/_ro/trn_rl_repo"):
    if os.path.isdir(_p) and _p not in sys.path:
        sys.path.append(_p)

from contextlib import ExitStack

import numpy as np

import concourse.bass as bass
import concourse.tile as tile
from concourse import bacc, mybir
from concourse.bass_utils import run_bass_kernel_spmd

F32 = mybir.dt.float32
BF16 = mybir.dt.bfloat16
P = 128
CHUNK = 512
DIAG = CHUNK // P

S, E, NHEAD = 2048, 2048, 16
BATCH = 2
H = 4
NJ = 3 * H
NQK = 2 * H
EB = E // P
SC = S // CHUNK
SB = S // P
EC = E // CHUNK
N_CORES = 8


def _emit(nc):
    xT = nc.dram_tensor("xT", [E, S], BF16, kind="ExternalInput").ap()
    wqkv = nc.dram_tensor("wqkv", [E, NJ * P], BF16, kind="ExternalInput").ap()
    bqkv = nc.dram_tensor("bqkv", [P, NJ], F32, kind="ExternalInput").ap()
    wproj = nc.dram_tensor("wproj", [H * P, E], BF16, kind="ExternalInput").ap()
    tri = nc.dram_tensor("tri", [P, P], BF16, kind="ExternalInput").ap()
    ones = nc.dram_tensor("ones", [P, P], BF16, kind="ExternalInput").ap()
    bv = nc.dram_tensor("bv", [P, H * P], F32, kind="ExternalInput").ap()
    y = nc.dram_tensor("y", [S, E], BF16, kind="ExternalOutput").ap()

    xT_d = xT.rearrange("(eb p) s -> eb p s", p=P)
    wqkv_d = wqkv.rearrange("(eb p) j -> eb p j", p=P)
    wproj_d = wproj.rearrange("(hb p) e -> hb p e", p=P)

    with tile.TileContext(nc) as tc, ExitStack() as ctx:
        const = ctx.enter_context(tc.tile_pool(name="const", bufs=1))
        qkvT_pool = ctx.enter_context(tc.tile_pool(name="qkvT", bufs=1))
        vnat_pool = ctx.enter_context(tc.tile_pool(name="vnat", bufs=1))
        outT_pool = ctx.enter_context(tc.tile_pool(name="outT", bufs=1))
        # PSUM: accum 2 + scores 2 + av-out 2 + p3/den 2 = 8 banks
        psum_k = ctx.enter_context(tc.tile_pool(name="psum_k", bufs=2, space="PSUM"))
        psum_s = ctx.enter_context(tc.tile_pool(name="psum_s", bufs=2, space="PSUM"))
        psum_o = ctx.enter_context(tc.tile_pool(name="psum_o", bufs=2, space="PSUM"))
        psum_p = ctx.enter_context(tc.tile_pool(name="psum_p", bufs=2, space="PSUM"))
        exp_pool = ctx.enter_context(tc.tile_pool(name="exp", bufs=6))
        acc_pool = ctx.enter_context(tc.tile_pool(name="acc", bufs=12))
        recip_pool = ctx.enter_context(tc.tile_pool(name="recip", bufs=1))

        bq_t = const.tile([P, NJ], F32)
        tri_t = const.tile([P, P], BF16)
        ones_t = const.tile([P, P], BF16)
        bv_t = const.tile([P, H * P], F32)

        qkT = [qkvT_pool.tile([P, S], BF16, name=f"qkT{jb}") for jb in range(NQK)]
        vnat = [vnat_pool.tile([P, H * P], BF16, name=f"vn{sb}") for sb in range(SB)]
        outT = [outT_pool.tile([P, S], BF16, name=f"outT{h}") for h in range(H)]

        # ---- phase-2 unit emitter (one attention (h, ci) unit), yielding
        # after each (scores, exp, av, tree) block so it can be sliced into
        # the phase-1 instruction streams ----
        def p2_chunk_feeder(ci):
            i0 = ci * CHUNK
            njb = (ci + 1) * DIAG
            for h in range(H):
                qT, kT = qkT[h], qkT[H + h]
                out_ps = psum_o.tile([P, CHUNK], F32, name="ps_o")
                exs = []
                lvl = []
                tA = None
                for jb in range(njb):
                    dt = jb - DIAG * ci
                    off = max(dt, 0) * P
                    w = CHUNK - off
                    sc_ps = psum_s.tile([P, CHUNK], F32, name="ps_s")
                    nc.tensor.matmul(
                        sc_ps[:, 0:w],
                        kT[:, jb * P : (jb + 1) * P],
                        qT[:, i0 + off : i0 + CHUNK],
                        start=True,
                        stop=True,
                    )
                    ex = exp_pool.tile([P, CHUNK], BF16, name="ex")
                    nc.scalar.activation(
                        ex[:, 0:w], sc_ps[:, 0:w], mybir.ActivationFunctionType.Exp
                    )
                    if dt >= 0:
                        nc.vector.tensor_mul(ex[:, 0:P], ex[:, 0:P], tri_t[:])
                    nc.tensor.matmul(
                        out_ps[:, off:CHUNK],
                        vnat[jb][:, h * P : (h + 1) * P],
                        ex[:, 0:w],
                        start=(jb == 0),
                        stop=(jb == njb - 1),
                    )
                    exs.append(ex)
                    if jb < njb - 4 and jb % 2 == 1:
                        t = acc_pool.tile([P, CHUNK], BF16, name="acc")
                        nc.vector.tensor_add(t[:], exs[jb - 1][:], exs[jb][:])
                        lvl.append(t)
                    elif jb == njb - 3:
                        d0, d1 = exs[jb - 1], exs[jb]
                        tA = acc_pool.tile([P, CHUNK], BF16, name="acc")
                        nc.vector.tensor_add(
                            tA[:, P:CHUNK], d0[:, P:CHUNK], d1[:, 0 : CHUNK - P]
                        )
                        nc.vector.tensor_copy(tA[:, 0:P], d0[:, 0:P])
                    elif jb == njb - 1:
                        d2, d3 = exs[jb - 1], exs[jb]
                        tB = acc_pool.tile([P, CHUNK], BF16, name="acc")
                        nc.vector.tensor_add(
                            tB[:, 3 * P : CHUNK], d2[:, P : 2 * P], d3[:, 0:P]
                        )
                        nc.vector.tensor_copy(tB[:, 2 * P : 3 * P], d2[:, 0:P])
                        tD = acc_pool.tile([P, CHUNK], BF16, name="acc")
                        nc.vector.tensor_add(
                            tD[:, 2 * P : CHUNK],
                            tA[:, 2 * P : CHUNK],
                            tB[:, 2 * P : CHUNK],
                        )
                        nc.vector.tensor_copy(tD[:, 0 : 2 * P], tA[:, 0 : 2 * P])
                        lvl.append(tD)
                    yield w
                while len(lvl) > 1:
                    nxt = []
                    for p in range(0, len(lvl) - 1, 2):
                        t = acc_pool.tile([P, CHUNK], BF16, name="acc")
                        nc.vector.tensor_add(t[:], lvl[p][:], lvl[p + 1][:])
                        nxt.append(t)
                    if len(lvl) % 2:
                        nxt.append(lvl[-1])
                    lvl = nxt
                den_ps = psum_p.tile([P, CHUNK], F32, name="ps_p")
                nc.tensor.matmul(den_ps[:], ones_t[:], lvl[0][:], start=True, stop=True)
                rc = recip_pool.tile([P, CHUNK], F32, name="rc")
                nc.vector.reciprocal_approx_fast(rc[:], den_ps[:])
                nc.vector.tensor_mul(outT[h][:, i0 : i0 + CHUNK], out_ps[:], rc[:])

        with tc.tile_pool(name="wq", bufs=1) as wq_pool, tc.tile_pool(
            name="xTs", bufs=1
        ) as xT_pool:
            # ---- streamed weight/xT loads ----
            wq_tiles = []
            xT_tiles = []
            dma_q = [nc.sync, nc.scalar, nc.gpsimd]
            crit_q = [
                nc.gpsimd if eb % 2 == 1 else (nc.sync if eb % 4 == 0 else nc.scalar)
                for eb in range(EB)
            ]
            for eb in range(EB):
                q = crit_q[eb]
                xt = xT_pool.tile([P, S], BF16, name=f"xT{eb}")
                (nc.scalar if eb == 0 else q).dma_start(
                    xt[:, 0:CHUNK], xT_d[eb][:, 0:CHUNK]
                )
                xT_tiles.append(xt)
                t = wq_pool.tile([P, NJ * P], BF16, name=f"wq{eb}")
                if eb < 4:
                    q.dma_start(t[:, 0:P], wqkv_d[eb][:, 0:P])
                    q.dma_start(t[:, P : NQK * P], wqkv_d[eb][:, P : NQK * P])
                else:
                    q.dma_start(t[:, 0 : NQK * P], wqkv_d[eb][:, 0 : NQK * P])
                wq_tiles.append(t)
            for eb in range(EB):
                crit_q[eb].dma_start(
                    wq_tiles[eb][:, NQK * P : NJ * P], wqkv_d[eb][:, NQK * P : NJ * P]
                )
            nc.sync.dma_start(bq_t[:], bqkv[:])
            nc.scalar.dma_start(bv_t[:], bv[:])
            nc.sync.dma_start(tri_t[:], tri[:])
            nc.scalar.dma_start(ones_t[:], ones[:])
            qi = 0
            for sc in range(1, SC):
                s0 = sc * CHUNK
                for eb in range(EB):
                    dma_q[qi % 3].dma_start(
                        xT_tiles[eb][:, s0 : s0 + CHUNK], xT_d[eb][:, s0 : s0 + CHUNK]
                    )
                    qi += 1

            # ---- phase 1 chunk 0: eb-major across all 8 banks (paced with
            # the streaming loads) ----
            arrival = [0, 1, 2, 3, 5, 4, 6, 7, 9, 8, 10, 11, 13, 12, 14, 15]
            ps = [psum_k.tile([P, CHUNK], F32, name="ps_k") for _ in range(2)]
            ps += [psum_s.tile([P, CHUNK], F32, name="ps_s") for _ in range(2)]
            ps += [psum_o.tile([P, CHUNK], F32, name="ps_o") for _ in range(2)]
            ps += [psum_p.tile([P, CHUNK], F32, name="ps_p") for _ in range(2)]
            for k, eb in enumerate(arrival):
                for jb in range(NQK):
                    nc.tensor.matmul(
                        ps[jb][:],
                        wq_tiles[eb][:, jb * P : (jb + 1) * P],
                        xT_tiles[eb][:, 0:CHUNK],
                        start=(k == 0),
                        stop=(k == EB - 1),
                    )
            for jb in range(NQK):
                nc.vector.tensor_scalar_add(
                    qkT[jb][:, 0:CHUNK], ps[jb][:], bq_t[:, jb : jb + 1]
                )
            for r in range(DIAG):
                pv = psum_k.tile([P, H * P], F32, name="ps_k")
                for eb in range(EB):
                    nc.tensor.matmul(
                        pv[:],
                        xT_tiles[eb][:, r * P : (r + 1) * P],
                        wq_tiles[eb][:, NQK * P : NJ * P],
                        start=(eb == 0),
                        stop=(eb == EB - 1),
                    )
                nc.vector.tensor_add(vnat[r][:], pv[:], bv_t[:])

            # ---- phase 1 chunks 1..3: jb-major, with chunk ci=sc-1's
            # attention units sliced into the stream ----
            for sc in range(1, SC):
                s0 = sc * CHUNK
                feeder = p2_chunk_feeder(sc - 1)
                per_slot = (16 * sc + 11) // 12
                for jb in range(NQK):
                    pk = psum_k.tile([P, CHUNK], F32, name="ps_k")
                    for eb in range(EB):
                        nc.tensor.matmul(
                            pk[:],
                            wq_tiles[eb][:, jb * P : (jb + 1) * P],
                            xT_tiles[eb][:, s0 : s0 + CHUNK],
                            start=(eb == 0),
                            stop=(eb == EB - 1),
                        )
                    nc.vector.tensor_scalar_add(
                        qkT[jb][:, s0 : s0 + CHUNK], pk[:], bq_t[:, jb : jb + 1]
                    )
                    for _ in range(per_slot):
                        next(feeder, None)
                for r in range(DIAG):
                    sb = sc * DIAG + r
                    pv = psum_k.tile([P, H * P], F32, name="ps_k")
                    for eb in range(EB):
                        nc.tensor.matmul(
                            pv[:],
                            xT_tiles[eb][:, s0 + r * P : s0 + (r + 1) * P],
                            wq_tiles[eb][:, NQK * P : NJ * P],
                            start=(eb == 0),
                            stop=(eb == EB - 1),
                        )
                    nc.vector.tensor_add(vnat[sb][:], pv[:], bv_t[:])
                    for _ in range(per_slot):
                        next(feeder, None)
                for _ in feeder:  # finish any remainder of the chunk's units
                    pass

        # ---- post-phase-1: wp loads, chunk-3 attention, all of c_proj ----
        wp_pool = ctx.enter_context(tc.tile_pool(name="wp", bufs=1))
        yout_pool = ctx.enter_context(tc.tile_pool(name="yout", bufs=2))
        wp_tiles = []
        for hb in range(H):
            t = wp_pool.tile([P, E], BF16, name=f"wp{hb}")
            dma_q[hb % 3].dma_start(t[:], wproj_d[hb])
            wp_tiles.append(t)

        def p3_group_gen(ci, tail=False):
            for r in range(DIAG):
                sb = ci * DIAG + r
                ot = yout_pool.tile([P, E], BF16, name="yo")
                for ec in range(EC):
                    ps3 = psum_p.tile([P, CHUNK], F32, name="ps_p")
                    for h in range(H):
                        nc.tensor.matmul(
                            ps3[:],
                            outT[h][:, sb * P : (sb + 1) * P],
                            wp_tiles[h][:, ec * CHUNK : (ec + 1) * CHUNK],
                            start=(h == 0),
                            stop=(h == H - 1),
                        )
                    if (tail and ec % 2 == 0) or (not tail and (sb + ec) % 4 == 0):
                        nc.scalar.copy(ot[:, ec * CHUNK : (ec + 1) * CHUNK], ps3[:])
                    else:
                        nc.vector.tensor_copy(
                            ot[:, ec * CHUNK : (ec + 1) * CHUNK], ps3[:]
                        )
                    if sb == SB - 1:
                        (nc.gpsimd if ec % 2 == 0 else nc.sync).dma_start(
                            y[sb * P : (sb + 1) * P, ec * CHUNK : (ec + 1) * CHUNK],
                            ot[:, ec * CHUNK : (ec + 1) * CHUNK],
                        )
                yield
                if sb < SB - 1:
                    [nc.sync, nc.gpsimd][sb % 2].dma_start(
                        y[sb * P : (sb + 1) * P, :], ot[:]
                    )

        # chunk-3 attention with c_proj groups (chunks 0-2) as deficit-paced
        # PE filler, then the remaining groups as the tail
        def p3_all():
            for ci in range(SC):
                yield from p3_group_gen(ci, tail=(ci == SC - 1))

        filler = p3_all()
        feeder = p2_chunk_feeder(SC - 1)
        deficit = 0.0
        for w in feeder:
            deficit += (0.15 + w * 0.00098) - w * 0.000834
            if deficit >= 3.44:  # one c_proj row (4 groups) per fill
                next(filler, None)
                deficit -= 3.44
        for _ in filler:
            pass
    return nc


_NC = None
LAST_RESULTS = None


def _get_nc():
    global _NC
    if _NC is None:
        nc = bacc.Bacc(
            "TRN2", target_bir_lowering=False, debug=False, num_devices=N_CORES
        )
        _emit(nc)
        nc.compile()
        _NC = nc
    return _NC


def _prep_shared(hidden_states, c_attn_w, c_attn_b, c_proj_w):
    import ml_dtypes

    bf16 = ml_dtypes.bfloat16
    scale = 1.0 / float(np.sqrt(P))
    xT = [
        np.ascontiguousarray(hidden_states[b].T).astype(bf16) for b in range(BATCH)
    ]
    pp = np.arange(P)
    tri = (pp[:, None] <= pp[None, :]).astype(bf16)
    ones = np.ones((P, P), dtype=bf16)
    return xT, tri, ones, scale, bf16


def _core_inputs(shared, c_attn_w, c_attn_b, c_proj_w, core):
    xT, tri, ones, scale, bf16 = shared
    b, g = core // 4, core % 4
    h0 = H * g
    cols = []
    for part in range(3):
        for h in range(h0, h0 + H):
            base = part * E + h * P
            cols.extend(range(base, base + P))
    cols = np.asarray(cols)
    wqkv = np.ascontiguousarray(c_attn_w[:, cols]).astype(np.float32)
    bq = np.ascontiguousarray(c_attn_b[cols]).astype(np.float32)
    wqkv[:, 0 : H * P] *= scale
    bq[0 : H * P] *= scale
    bq = bq.reshape(NJ, P).T.copy()
    wproj = np.ascontiguousarray(c_proj_w[h0 * P : (h0 + H) * P, :]).astype(bf16)
    bv = np.broadcast_to(bq[:, NQK:NJ].T.reshape(1, H * P), (P, H * P)).copy()
    return {
        "xT": xT[b],
        "wqkv": wqkv.astype(bf16),
        "bqkv": bq,
        "wproj": wproj,
        "tri": tri,
        "ones": ones,
        "bv": bv,
    }


def kernel(hidden_states, c_attn_w, c_attn_b, c_proj_w, c_proj_b):
    global LAST_RESULTS
    hidden_states = np.asarray(hidden_states)
    c_attn_w = np.asarray(c_attn_w)
    c_attn_b = np.asarray(c_attn_b)
    c_proj_w = np.asarray(c_proj_w)
    c_proj_b = np.asarray(c_proj_b)

    nc = _get_nc()
    shared = _prep_shared(hidden_states, c_attn_w, c_attn_b, c_proj_w)
    in_maps = [
        _core_inputs(shared, c_attn_w, c_attn_b, c_proj_w, c)
        for c in range(N_CORES)
    ]
    res = run_bass_kernel_spmd(nc, in_maps, list(range(N_CORES)))
    LAST_RESULTS = res
    out = np.zeros((BATCH, S, E), dtype=np.float32)
    for c in range(N_CORES):
        out[c // 4] += res.results[c]["y"].astype(np.float32)
    out += c_proj_b.astype(np.float32)[None, None, :]
    return out


# revision 4
# speedup vs baseline: 1.0405x; 1.0201x over previous
"""nn_Attention — tensor-parallel causal attention on 8 TRN2 NeuronCores. v8.

vs v7 (333us): phase-1 chunks sc>=1 run the qk projection jb-major (one
PSUM bank per j-block accumulated over all 16 ebs) instead of eb-major,
freeing 6 PSUM banks. The phase-2 attention units of chunk ci=sc-1 are
emitted interleaved into the phase-1 streams (engines execute in emission
order): ScalarE exp for chunks 0-2 is prepaid inside the phase-1 window
where ScalarE idles, so the post-phase-1 stretch (chunk-3 attention +
all c_proj) is PE-bound instead of exp-bound. Denominator row-sums stay
on the DVE bf16 tree + one N=512 ones-matmul per (head, chunk).
"""

import os
import sys

for _p in ("/opt/trn_rl_repo", "/root/.axon_site/_ro/trn_rl_repo"):
    if os.path.isdir(_p) and _p not in sys.path:
        sys.path.append(_p)

from contextlib import ExitStack

import numpy as np

import concourse.bass as bass
import concourse.tile as tile
from concourse import bacc, mybir
from concourse.bass_utils import run_bass_kernel_spmd

F32 = mybir.dt.float32
BF16 = mybir.dt.bfloat16
P = 128
CHUNK = 512
DIAG = CHUNK // P

S, E, NHEAD = 2048, 2048, 16
BATCH = 2
H = 4
NJ = 3 * H
NQK = 2 * H
EB = E // P
SC = S // CHUNK
SB = S // P
EC = E // CHUNK
N_CORES = 8


def _emit(nc):
    xT = nc.dram_tensor("xT", [E, S], BF16, kind="ExternalInput").ap()
    wqkv = nc.dram_tensor("wqkv", [E, NJ * P], BF16, kind="ExternalInput").ap()
    bqkv = nc.dram_tensor("bqkv", [P, NJ], F32, kind="ExternalInput").ap()
    wproj = nc.dram_tensor("wproj", [H * P, E], BF16, kind="ExternalInput").ap()
    tri = nc.dram_tensor("tri", [P, P], BF16, kind="ExternalInput").ap()
    ones = nc.dram_tensor("ones", [P, P], BF16, kind="ExternalInput").ap()
    bv = nc.dram_tensor("bv", [P, H * P], F32, kind="ExternalInput").ap()
    y = nc.dram_tensor("y", [S, E], BF16, kind="ExternalOutput").ap()

    xT_d = xT.rearrange("(eb p) s -> eb p s", p=P)
    wqkv_d = wqkv.rearrange("(eb p) j -> eb p j", p=P)
    wproj_d = wproj.rearrange("(hb p) e -> hb p e", p=P)

    with tile.TileContext(nc) as tc, ExitStack() as ctx:
        const = ctx.enter_context(tc.tile_pool(name="const", bufs=1))
        qkvT_pool = ctx.enter_context(tc.tile_pool(name="qkvT", bufs=1))
        vnat_pool = ctx.enter_context(tc.tile_pool(name="vnat", bufs=1))
        outT_pool = ctx.enter_context(tc.tile_pool(name="outT", bufs=1))
        # PSUM: accum 2 + score-pairs 2x2 + av-out/den 2 = 8 banks
        # (phase-3 reuses the accum banks after phase 1)
        psum_k = ctx.enter_context(tc.tile_pool(name="psum_k", bufs=2, space="PSUM"))
        psum_s2 = ctx.enter_context(tc.tile_pool(name="psum_s2", bufs=2, space="PSUM"))
        psum_o = ctx.enter_context(tc.tile_pool(name="psum_o", bufs=2, space="PSUM"))
        exp_pool = ctx.enter_context(tc.tile_pool(name="exp", bufs=6))
        acc_pool = ctx.enter_context(tc.tile_pool(name="acc", bufs=12))
        recip_pool = ctx.enter_context(tc.tile_pool(name="recip", bufs=1))

        bq_t = const.tile([P, NJ], F32)
        tri_t = const.tile([P, P], BF16)
        ones_t = const.tile([P, P], BF16)
        bv_t = const.tile([P, H * P], F32)

        qkT = [qkvT_pool.tile([P, S], BF16, name=f"qkT{jb}") for jb in range(NQK)]
        vnat = [vnat_pool.tile([P, H * P], BF16, name=f"vn{sb}") for sb in range(SB)]
        outT = [outT_pool.tile([P, S], BF16, name=f"outT{h}") for h in range(H)]

        # ---- phase-2 unit emitter (one attention (h, ci) unit), yielding
        # after each (scores, exp, av, tree) block so it can be sliced into
        # the phase-1 instruction streams ----
        def p2_chunk_feeder(ci):
            i0 = ci * CHUNK
            njb = (ci + 1) * DIAG
            npair = njb // 2
            for h in range(H):
                qT, kT = qkT[h], qkT[H + h]
                out_ps = psum_o.tile([P, CHUNK], F32, name="ps_o")
                lvl = []
                tA = None
                for p in range(npair):
                    jb0, jb1 = 2 * p, 2 * p + 1
                    ps2 = psum_s2.tile([P, 2 * CHUNK], F32, name="ps_s2")
                    ex2 = exp_pool.tile([P, 2 * CHUNK], BF16, name="ex")
                    if jb0 == njb - 4:  # diagonal pair (dt0 w=512, dt1 w=384)
                        nc.tensor.matmul(
                            ps2[:, 0:CHUNK], kT[:, jb0 * P : (jb0 + 1) * P],
                            qT[:, i0 : i0 + CHUNK], start=True, stop=True)
                        nc.tensor.matmul(
                            ps2[:, CHUNK : CHUNK + 3 * P],
                            kT[:, jb1 * P : (jb1 + 1) * P],
                            qT[:, i0 + P : i0 + CHUNK], start=True, stop=True)
                        nc.scalar.activation(
                            ex2[:, 0 : CHUNK + 3 * P], ps2[:, 0 : CHUNK + 3 * P],
                            mybir.ActivationFunctionType.Exp)
                        nc.vector.tensor_mul(ex2[:, 0:P], ex2[:, 0:P], tri_t[:])
                        nc.vector.tensor_mul(
                            ex2[:, CHUNK : CHUNK + P], ex2[:, CHUNK : CHUNK + P],
                            tri_t[:])
                        nc.tensor.matmul(
                            out_ps[:, 0:CHUNK], vnat[jb0][:, h * P : (h + 1) * P],
                            ex2[:, 0:CHUNK], start=(p == 0), stop=False)
                        nc.tensor.matmul(
                            out_ps[:, P:CHUNK], vnat[jb1][:, h * P : (h + 1) * P],
                            ex2[:, CHUNK : CHUNK + 3 * P], start=False, stop=False)
                        tA = acc_pool.tile([P, CHUNK], BF16, name="acc")
                        nc.vector.tensor_add(
                            tA[:, P:CHUNK], ex2[:, P:CHUNK],
                            ex2[:, CHUNK : CHUNK + 3 * P])
                        nc.vector.tensor_copy(tA[:, 0:P], ex2[:, 0:P])
                        yield 896
                    elif jb0 == njb - 2:  # diagonal pair (dt2 w=256, dt3 w=128)
                        nc.tensor.matmul(
                            ps2[:, 0 : 2 * P], kT[:, jb0 * P : (jb0 + 1) * P],
                            qT[:, i0 + 2 * P : i0 + CHUNK], start=True, stop=True)
                        nc.tensor.matmul(
                            ps2[:, 2 * P : 3 * P], kT[:, jb1 * P : (jb1 + 1) * P],
                            qT[:, i0 + 3 * P : i0 + CHUNK], start=True, stop=True)
                        nc.scalar.activation(
                            ex2[:, 0 : 3 * P], ps2[:, 0 : 3 * P],
                            mybir.ActivationFunctionType.Exp)
                        nc.vector.tensor_mul(ex2[:, 0:P], ex2[:, 0:P], tri_t[:])
                        nc.vector.tensor_mul(
                            ex2[:, 2 * P : 3 * P], ex2[:, 2 * P : 3 * P], tri_t[:])
                        nc.tensor.matmul(
                            out_ps[:, 2 * P : CHUNK],
                            vnat[jb0][:, h * P : (h + 1) * P], ex2[:, 0 : 2 * P],
                            start=False, stop=False)
                        nc.tensor.matmul(
                            out_ps[:, 3 * P : CHUNK],
                            vnat[jb1][:, h * P : (h + 1) * P], ex2[:, 2 * P : 3 * P],
                            start=False, stop=True)
                        tB = acc_pool.tile([P, CHUNK], BF16, name="acc")
                        nc.vector.tensor_add(
                            tB[:, 3 * P : CHUNK], ex2[:, P : 2 * P],
                            ex2[:, 2 * P : 3 * P])
                        nc.vector.tensor_copy(tB[:, 2 * P : 3 * P], ex2[:, 0:P])
                        tD = acc_pool.tile([P, CHUNK], BF16, name="acc")
                        nc.vector.tensor_add(
                            tD[:, 2 * P : CHUNK], tA[:, 2 * P : CHUNK],
                            tB[:, 2 * P : CHUNK])
                        nc.vector.tensor_copy(tD[:, 0 : 2 * P], tA[:, 0 : 2 * P])
                        lvl.append(tD)
                        yield 384
                    else:  # full off-diagonal pair
                        nc.tensor.matmul(
                            ps2[:, 0:CHUNK], kT[:, jb0 * P : (jb0 + 1) * P],
                            qT[:, i0 : i0 + CHUNK], start=True, stop=True)
                        nc.tensor.matmul(
                            ps2[:, CHUNK : 2 * CHUNK], kT[:, jb1 * P : (jb1 + 1) * P],
                            qT[:, i0 : i0 + CHUNK], start=True, stop=True)
                        nc.scalar.activation(
                            ex2[:, 0 : 2 * CHUNK], ps2[:, 0 : 2 * CHUNK],
                            mybir.ActivationFunctionType.Exp)
                        nc.tensor.matmul(
                            out_ps[:, 0:CHUNK], vnat[jb0][:, h * P : (h + 1) * P],
                            ex2[:, 0:CHUNK], start=(p == 0), stop=False)
                        nc.tensor.matmul(
                            out_ps[:, 0:CHUNK], vnat[jb1][:, h * P : (h + 1) * P],
                            ex2[:, CHUNK : 2 * CHUNK], start=False, stop=False)
                        t = acc_pool.tile([P, CHUNK], BF16, name="acc")
                        nc.vector.tensor_add(
                            t[:], ex2[:, 0:CHUNK], ex2[:, CHUNK : 2 * CHUNK])
                        lvl.append(t)
                        yield 1024
                while len(lvl) > 1:
                    nxt = []
                    for p in range(0, len(lvl) - 1, 2):
                        t = acc_pool.tile([P, CHUNK], BF16, name="acc")
                        nc.vector.tensor_add(t[:], lvl[p][:], lvl[p + 1][:])
                        nxt.append(t)
                    if len(lvl) % 2:
                        nxt.append(lvl[-1])
                    lvl = nxt
                den_ps = psum_o.tile([P, CHUNK], F32, name="ps_o")
                nc.tensor.matmul(den_ps[:], ones_t[:], lvl[0][:], start=True, stop=True)
                rc = recip_pool.tile([P, CHUNK], F32, name="rc")
                nc.vector.reciprocal_approx_fast(rc[:], den_ps[:])
                nc.vector.tensor_mul(outT[h][:, i0 : i0 + CHUNK], out_ps[:], rc[:])

        with tc.tile_pool(name="wq", bufs=1) as wq_pool, tc.tile_pool(
            name="xTs", bufs=1
        ) as xT_pool:
            # ---- streamed weight/xT loads ----
            wq_tiles = []
            xT_tiles = []
            dma_q = [nc.sync, nc.scalar, nc.gpsimd]
            crit_q = [
                nc.gpsimd if (eb % 2 == 1 and eb != 1) else (
                    nc.sync if eb % 4 == 0 else nc.scalar)
                for eb in range(EB)
            ]
            for eb in range(EB):
                q = crit_q[eb]
                xt = xT_pool.tile([P, S], BF16, name=f"xT{eb}")
                (nc.scalar if eb == 0 else q).dma_start(
                    xt[:, 0:CHUNK], xT_d[eb][:, 0:CHUNK]
                )
                xT_tiles.append(xt)
                t = wq_pool.tile([P, NJ * P], BF16, name=f"wq{eb}")
                if eb < 4:
                    q.dma_start(t[:, 0:P], wqkv_d[eb][:, 0:P])
                    q.dma_start(t[:, P : NQK * P], wqkv_d[eb][:, P : NQK * P])
                else:
                    q.dma_start(t[:, 0 : NQK * P], wqkv_d[eb][:, 0 : NQK * P])
                wq_tiles.append(t)
            for eb in range(EB):
                crit_q[eb].dma_start(
                    wq_tiles[eb][:, NQK * P : NJ * P], wqkv_d[eb][:, NQK * P : NJ * P]
                )
            nc.sync.dma_start(bq_t[:], bqkv[:])
            nc.scalar.dma_start(bv_t[:], bv[:])
            nc.sync.dma_start(tri_t[:], tri[:])
            nc.scalar.dma_start(ones_t[:], ones[:])
            qi = 0
            for sc in range(1, SC):
                s0 = sc * CHUNK
                for eb in range(EB):
                    dma_q[qi % 3].dma_start(
                        xT_tiles[eb][:, s0 : s0 + CHUNK], xT_d[eb][:, s0 : s0 + CHUNK]
                    )
                    qi += 1

            # ---- phase 1 chunk 0: eb-major across all 8 banks (paced with
            # the streaming loads) ----
            arrival = [0, 1, 3, 2, 5, 4, 6, 7, 9, 8, 10, 11, 13, 12, 14, 15]
            ps = [psum_k.tile([P, CHUNK], F32, name="ps_k") for _ in range(2)]
            ps += [psum_o.tile([P, CHUNK], F32, name="ps_o") for _ in range(2)]
            pA = psum_s2.tile([P, 2 * CHUNK], F32, name="ps_s2")
            pB = psum_s2.tile([P, 2 * CHUNK], F32, name="ps_s2")
            ps += [pA[:, 0:CHUNK], pA[:, CHUNK:], pB[:, 0:CHUNK], pB[:, CHUNK:]]
            for k, eb in enumerate(arrival):
                for jb in range(NQK):
                    nc.tensor.matmul(
                        ps[jb][:],
                        wq_tiles[eb][:, jb * P : (jb + 1) * P],
                        xT_tiles[eb][:, 0:CHUNK],
                        start=(k == 0),
                        stop=(k == EB - 1),
                    )
            for jb in range(NQK):
                nc.vector.tensor_scalar_add(
                    qkT[jb][:, 0:CHUNK], ps[jb][:], bq_t[:, jb : jb + 1]
                )
            for r in range(DIAG):
                pv = psum_k.tile([P, H * P], F32, name="ps_k")
                for eb in range(EB):
                    nc.tensor.matmul(
                        pv[:],
                        xT_tiles[eb][:, r * P : (r + 1) * P],
                        wq_tiles[eb][:, NQK * P : NJ * P],
                        start=(eb == 0),
                        stop=(eb == EB - 1),
                    )
                nc.vector.tensor_add(vnat[r][:], pv[:], bv_t[:])

            # ---- phase 1 chunks 1..3: jb-major, with chunk ci=sc-1's
            # attention units sliced into the stream ----
            for sc in range(1, SC):
                s0 = sc * CHUNK
                feeder = p2_chunk_feeder(sc - 1)
                per_slot = (16 * sc + 11) // 12
                for jb in range(NQK):
                    pk = psum_k.tile([P, CHUNK], F32, name="ps_k")
                    for eb in range(EB):
                        nc.tensor.matmul(
                            pk[:],
                            wq_tiles[eb][:, jb * P : (jb + 1) * P],
                            xT_tiles[eb][:, s0 : s0 + CHUNK],
                            start=(eb == 0),
                            stop=(eb == EB - 1),
                        )
                    nc.vector.tensor_scalar_add(
                        qkT[jb][:, s0 : s0 + CHUNK], pk[:], bq_t[:, jb : jb + 1]
                    )
                    for _ in range(per_slot):
                        next(feeder, None)
                for r in range(DIAG):
                    sb = sc * DIAG + r
                    pv = psum_k.tile([P, H * P], F32, name="ps_k")
                    for eb in range(EB):
                        nc.tensor.matmul(
                            pv[:],
                            xT_tiles[eb][:, s0 + r * P : s0 + (r + 1) * P],
                            wq_tiles[eb][:, NQK * P : NJ * P],
                            start=(eb == 0),
                            stop=(eb == EB - 1),
                        )
                    nc.vector.tensor_add(vnat[sb][:], pv[:], bv_t[:])
                    for _ in range(per_slot):
                        next(feeder, None)
                for _ in feeder:  # finish any remainder of the chunk's units
                    pass

        # ---- post-phase-1: wp loads, chunk-3 attention, all of c_proj ----
        wp_pool = ctx.enter_context(tc.tile_pool(name="wp", bufs=1))
        yout_pool = ctx.enter_context(tc.tile_pool(name="yout", bufs=2))
        wp_tiles = []
        for hb in range(H):
            t = wp_pool.tile([P, E], BF16, name=f"wp{hb}")
            dma_q[hb % 3].dma_start(t[:], wproj_d[hb])
            wp_tiles.append(t)

        def p3_group_gen(ci, tail=False):
            for r in range(DIAG):
                sb = ci * DIAG + r
                ot = yout_pool.tile([P, E], BF16, name="yo")
                for ec in range(EC):
                    ps3 = psum_k.tile([P, CHUNK], F32, name="ps_k")
                    for h in range(H):
                        nc.tensor.matmul(
                            ps3[:],
                            outT[h][:, sb * P : (sb + 1) * P],
                            wp_tiles[h][:, ec * CHUNK : (ec + 1) * CHUNK],
                            start=(h == 0),
                            stop=(h == H - 1),
                        )
                    if (tail and ec % 2 == 0) or (not tail and (sb + ec) % 4 == 0):
                        nc.scalar.copy(ot[:, ec * CHUNK : (ec + 1) * CHUNK], ps3[:])
                    else:
                        nc.vector.tensor_copy(
                            ot[:, ec * CHUNK : (ec + 1) * CHUNK], ps3[:]
                        )
                    if sb == SB - 1:
                        (nc.gpsimd if ec % 2 == 0 else nc.sync).dma_start(
                            y[sb * P : (sb + 1) * P, ec * CHUNK : (ec + 1) * CHUNK],
                            ot[:, ec * CHUNK : (ec + 1) * CHUNK],
                        )
                yield
                if sb < SB - 1:
                    [nc.sync, nc.gpsimd][sb % 2].dma_start(
                        y[sb * P : (sb + 1) * P, :], ot[:]
                    )

        # chunk-3 attention with c_proj groups (chunks 0-2) as deficit-paced
        # PE filler, then the remaining groups as the tail
        def p3_all():
            for ci in range(SC):
                yield from p3_group_gen(ci, tail=(ci == SC - 1))

        filler = p3_all()
        feeder = p2_chunk_feeder(SC - 1)
        deficit = 0.0
        for w in feeder:
            deficit += (0.15 + w * 0.00098) - w * 0.000834
            if deficit >= 2.6:  # one c_proj row (4 groups) per fill
                next(filler, None)
                deficit -= 3.44
        for _ in filler:
            pass
    return nc


_NC = None
LAST_RESULTS = None


def _get_nc():
    global _NC
    if _NC is None:
        nc = bacc.Bacc(
            "TRN2", target_bir_lowering=False, debug=False, num_devices=N_CORES
        )
        _emit(nc)
        nc.compile()
        _NC = nc
    return _NC


def _prep_shared(hidden_states, c_attn_w, c_attn_b, c_proj_w):
    import ml_dtypes

    bf16 = ml_dtypes.bfloat16
    scale = 1.0 / float(np.sqrt(P))
    xT = [
        np.ascontiguousarray(hidden_states[b].T).astype(bf16) for b in range(BATCH)
    ]
    pp = np.arange(P)
    tri = (pp[:, None] <= pp[None, :]).astype(bf16)
    ones = np.ones((P, P), dtype=bf16)
    return xT, tri, ones, scale, bf16


def _core_inputs(shared, c_attn_w, c_attn_b, c_proj_w, core):
    xT, tri, ones, scale, bf16 = shared
    b, g = core // 4, core % 4
    h0 = H * g
    cols = []
    for part in range(3):
        for h in range(h0, h0 + H):
            base = part * E + h * P
            cols.extend(range(base, base + P))
    cols = np.asarray(cols)
    wqkv = np.ascontiguousarray(c_attn_w[:, cols]).astype(np.float32)
    bq = np.ascontiguousarray(c_attn_b[cols]).astype(np.float32)
    wqkv[:, 0 : H * P] *= scale
    bq[0 : H * P] *= scale
    bq = bq.reshape(NJ, P).T.copy()
    wproj = np.ascontiguousarray(c_proj_w[h0 * P : (h0 + H) * P, :]).astype(bf16)
    bv = np.broadcast_to(bq[:, NQK:NJ].T.reshape(1, H * P), (P, H * P)).copy()
    return {
        "xT": xT[b],
        "wqkv": wqkv.astype(bf16),
        "bqkv": bq,
        "wproj": wproj,
        "tri": tri,
        "ones": ones,
        "bv": bv,
    }


def kernel(hidden_states, c_attn_w, c_attn_b, c_proj_w, c_proj_b):
    global LAST_RESULTS
    hidden_states = np.asarray(hidden_states)
    c_attn_w = np.asarray(c_attn_w)
    c_attn_b = np.asarray(c_attn_b)
    c_proj_w = np.asarray(c_proj_w)
    c_proj_b = np.asarray(c_proj_b)

    nc = _get_nc()
    shared = _prep_shared(hidden_states, c_attn_w, c_attn_b, c_proj_w)
    in_maps = [
        _core_inputs(shared, c_attn_w, c_attn_b, c_proj_w, c)
        for c in range(N_CORES)
    ]
    res = run_bass_kernel_spmd(nc, in_maps, list(range(N_CORES)))
    LAST_RESULTS = res
    out = np.zeros((BATCH, S, E), dtype=np.float32)
    for c in range(N_CORES):
        out[c // 4] += res.results[c]["y"].astype(np.float32)
    out += c_proj_b.astype(np.float32)[None, None, :]
    return out


# revision 5
# speedup vs baseline: 1.0502x; 1.0093x over previous
"""nn_Attention — tensor-parallel causal attention on 8 TRN2 NeuronCores. v9.

~321us (baseline 334us), rel_err 4.1e-3. Batch x head-group TP: core c ->
batch c//4, heads 4*(c%4)..; host pre-transposes/casts x, folds 1/sqrt(d)
into the q weights, sums the 4 c_proj partials per batch and adds biases.

Key structure (engines execute their instruction streams in emission
order, so overlap is programmed at the source level):
- Phase 1 chunk 0: eb-major qk accumulation across all 8 PSUM banks,
  paced with the streaming DMA loads (3 queues, jb0 slices split out so
  the first matmuls gate on 32KB).
- Phase 1 chunks 1..3: jb-major (one accum bank per j-block over all 16
  ebs) which needs only 2 banks; the attention units of chunk ci=sc-1
  are sliced INTO the phase-1 streams, prepaying ScalarE exp for chunks
  0-2 inside windows where ScalarE would idle.
- Attention blocks are emitted in PAIRS sharing one [128,1024] two-bank
  PSUM tile; ONE exp activation covers both blocks' contiguous valid
  range (80 activations instead of 160), including the diagonal pairs
  (w 512+384 -> exp[0:896]; w 256+128 packed in one bank -> exp[0:384]).
- Softmax denominators: bf16 pair/tree adds on the DVE (short dependency
  chains) + one N=512 ones-matmul per (head, chunk) broadcasting the row
  sums (3.4us of PE total vs 29us for per-block ones-matmuls).
- Post-phase-1: chunk-3 attention with c_proj row-groups injected as PE
  filler via an exp-vs-PE deficit model; c_proj groups reuse the then-
  idle phase-1 accum banks; drains split Scalar/DVE by measured load;
  y rows stream out on the sync/gpsimd queues as they complete.

fp8 DoubleRow (2x PE rate, confirmed by microbenchmark) was evaluated
and rejected: every placement (QKV / scores / av / c_proj / even the
denominator alone) exceeds the 2e-2 error budget per a host-side
numerics sim that exactly reproduces the hardware baseline error.
"""

import os
import sys

for _p in ("/opt/trn_rl_repo", "/root/.axon_site/_ro/trn_rl_repo"):
    if os.path.isdir(_p) and _p not in sys.path:
        sys.path.append(_p)

from contextlib import ExitStack

import numpy as np

import concourse.bass as bass
import concourse.tile as tile
from concourse import bacc, mybir
from concourse.bass_utils import run_bass_kernel_spmd

F32 = mybir.dt.float32
BF16 = mybir.dt.bfloat16
P = 128
CHUNK = 512
DIAG = CHUNK // P

S, E, NHEAD = 2048, 2048, 16
BATCH = 2
H = 4
NJ = 3 * H
NQK = 2 * H
EB = E // P
SC = S // CHUNK
SB = S // P
EC = E // CHUNK
N_CORES = 8


def _emit(nc):
    xT = nc.dram_tensor("xT", [E, S], BF16, kind="ExternalInput").ap()
    wqkv = nc.dram_tensor("wqkv", [E, NJ * P], BF16, kind="ExternalInput").ap()
    bqkv = nc.dram_tensor("bqkv", [P, NJ], F32, kind="ExternalInput").ap()
    wproj = nc.dram_tensor("wproj", [H * P, E], BF16, kind="ExternalInput").ap()
    tri = nc.dram_tensor("tri", [P, P], BF16, kind="ExternalInput").ap()
    ones = nc.dram_tensor("ones", [P, P], BF16, kind="ExternalInput").ap()
    bv = nc.dram_tensor("bv", [P, H * P], F32, kind="ExternalInput").ap()
    y = nc.dram_tensor("y", [S, E], BF16, kind="ExternalOutput").ap()

    xT_d = xT.rearrange("(eb p) s -> eb p s", p=P)
    wqkv_d = wqkv.rearrange("(eb p) j -> eb p j", p=P)
    wproj_d = wproj.rearrange("(hb p) e -> hb p e", p=P)

    with tile.TileContext(nc) as tc, ExitStack() as ctx:
        const = ctx.enter_context(tc.tile_pool(name="const", bufs=1))
        qkvT_pool = ctx.enter_context(tc.tile_pool(name="qkvT", bufs=1))
        vnat_pool = ctx.enter_context(tc.tile_pool(name="vnat", bufs=1))
        outT_pool = ctx.enter_context(tc.tile_pool(name="outT", bufs=1))
        # PSUM: accum 2 + score-pairs 2x2 + av-out/den 2 = 8 banks
        # (phase-3 reuses the accum banks after phase 1)
        psum_k = ctx.enter_context(tc.tile_pool(name="psum_k", bufs=2, space="PSUM"))
        psum_s2 = ctx.enter_context(tc.tile_pool(name="psum_s2", bufs=2, space="PSUM"))
        psum_o = ctx.enter_context(tc.tile_pool(name="psum_o", bufs=2, space="PSUM"))
        exp_pool = ctx.enter_context(tc.tile_pool(name="exp", bufs=6))
        acc_pool = ctx.enter_context(tc.tile_pool(name="acc", bufs=12))
        recip_pool = ctx.enter_context(tc.tile_pool(name="recip", bufs=1))

        bq_t = const.tile([P, NJ], F32)
        tri_t = const.tile([P, P], BF16)
        ones_t = const.tile([P, P], BF16)
        bv_t = const.tile([P, H * P], F32)

        qkT = [qkvT_pool.tile([P, S], BF16, name=f"qkT{jb}") for jb in range(NQK)]
        vnat = [vnat_pool.tile([P, H * P], BF16, name=f"vn{sb}") for sb in range(SB)]
        outT = [outT_pool.tile([P, S], BF16, name=f"outT{h}") for h in range(H)]

        # ---- phase-2 unit emitter (one attention (h, ci) unit), yielding
        # after each (scores, exp, av, tree) block so it can be sliced into
        # the phase-1 instruction streams ----
        def p2_chunk_feeder(ci):
            i0 = ci * CHUNK
            njb = (ci + 1) * DIAG
            npair = njb // 2
            for h in range(H):
                qT, kT = qkT[h], qkT[H + h]
                out_ps = psum_o.tile([P, CHUNK], F32, name="ps_o")
                lvl = []
                tA = None
                for p in range(npair):
                    jb0, jb1 = 2 * p, 2 * p + 1
                    ps2 = psum_s2.tile([P, 2 * CHUNK], F32, name="ps_s2")
                    ex2 = exp_pool.tile([P, 2 * CHUNK], BF16, name="ex")
                    if jb0 == njb - 4:  # diagonal pair (dt0 w=512, dt1 w=384)
                        nc.tensor.matmul(
                            ps2[:, 0:CHUNK], kT[:, jb0 * P : (jb0 + 1) * P],
                            qT[:, i0 : i0 + CHUNK], start=True, stop=True)
                        nc.tensor.matmul(
                            ps2[:, CHUNK : CHUNK + 3 * P],
                            kT[:, jb1 * P : (jb1 + 1) * P],
                            qT[:, i0 + P : i0 + CHUNK], start=True, stop=True)
                        nc.scalar.activation(
                            ex2[:, 0 : CHUNK + 3 * P], ps2[:, 0 : CHUNK + 3 * P],
                            mybir.ActivationFunctionType.Exp)
                        nc.vector.tensor_mul(ex2[:, 0:P], ex2[:, 0:P], tri_t[:])
                        nc.vector.tensor_mul(
                            ex2[:, CHUNK : CHUNK + P], ex2[:, CHUNK : CHUNK + P],
                            tri_t[:])
                        nc.tensor.matmul(
                            out_ps[:, 0:CHUNK], vnat[jb0][:, h * P : (h + 1) * P],
                            ex2[:, 0:CHUNK], start=(p == 0), stop=False)
                        nc.tensor.matmul(
                            out_ps[:, P:CHUNK], vnat[jb1][:, h * P : (h + 1) * P],
                            ex2[:, CHUNK : CHUNK + 3 * P], start=False, stop=False)
                        tA = acc_pool.tile([P, CHUNK], BF16, name="acc")
                        nc.vector.tensor_add(
                            tA[:, P:CHUNK], ex2[:, P:CHUNK],
                            ex2[:, CHUNK : CHUNK + 3 * P])
                        nc.vector.tensor_copy(tA[:, 0:P], ex2[:, 0:P])
                        yield 896
                    elif jb0 == njb - 2:  # diagonal pair (dt2 w=256, dt3 w=128)
                        nc.tensor.matmul(
                            ps2[:, 0 : 2 * P], kT[:, jb0 * P : (jb0 + 1) * P],
                            qT[:, i0 + 2 * P : i0 + CHUNK], start=True, stop=True)
                        nc.tensor.matmul(
                            ps2[:, 2 * P : 3 * P], kT[:, jb1 * P : (jb1 + 1) * P],
                            qT[:, i0 + 3 * P : i0 + CHUNK], start=True, stop=True)
                        nc.scalar.activation(
                            ex2[:, 0 : 3 * P], ps2[:, 0 : 3 * P],
                            mybir.ActivationFunctionType.Exp)
                        nc.vector.tensor_mul(ex2[:, 0:P], ex2[:, 0:P], tri_t[:])
                        nc.vector.tensor_mul(
                            ex2[:, 2 * P : 3 * P], ex2[:, 2 * P : 3 * P], tri_t[:])
                        nc.tensor.matmul(
                            out_ps[:, 2 * P : CHUNK],
                            vnat[jb0][:, h * P : (h + 1) * P], ex2[:, 0 : 2 * P],
                            start=False, stop=False)
                        nc.tensor.matmul(
                            out_ps[:, 3 * P : CHUNK],
                            vnat[jb1][:, h * P : (h + 1) * P], ex2[:, 2 * P : 3 * P],
                            start=False, stop=True)
                        tB = acc_pool.tile([P, CHUNK], BF16, name="acc")
                        nc.vector.tensor_add(
                            tB[:, 3 * P : CHUNK], ex2[:, P : 2 * P],
                            ex2[:, 2 * P : 3 * P])
                        nc.vector.tensor_copy(tB[:, 2 * P : 3 * P], ex2[:, 0:P])
                        tD = acc_pool.tile([P, CHUNK], BF16, name="acc")
                        nc.vector.tensor_add(
                            tD[:, 2 * P : CHUNK], tA[:, 2 * P : CHUNK],
                            tB[:, 2 * P : CHUNK])
                        nc.vector.tensor_copy(tD[:, 0 : 2 * P], tA[:, 0 : 2 * P])
                        lvl.append(tD)
                        yield 384
                    else:  # full off-diagonal pair
                        nc.tensor.matmul(
                            ps2[:, 0:CHUNK], kT[:, jb0 * P : (jb0 + 1) * P],
                            qT[:, i0 : i0 + CHUNK], start=True, stop=True)
                        nc.tensor.matmul(
                            ps2[:, CHUNK : 2 * CHUNK], kT[:, jb1 * P : (jb1 + 1) * P],
                            qT[:, i0 : i0 + CHUNK], start=True, stop=True)
                        nc.scalar.activation(
                            ex2[:, 0 : 2 * CHUNK], ps2[:, 0 : 2 * CHUNK],
                            mybir.ActivationFunctionType.Exp)
                        nc.tensor.matmul(
                            out_ps[:, 0:CHUNK], vnat[jb0][:, h * P : (h + 1) * P],
                            ex2[:, 0:CHUNK], start=(p == 0), stop=False)
                        nc.tensor.matmul(
                            out_ps[:, 0:CHUNK], vnat[jb1][:, h * P : (h + 1) * P],
                            ex2[:, CHUNK : 2 * CHUNK], start=False, stop=False)
                        t = acc_pool.tile([P, CHUNK], BF16, name="acc")
                        nc.vector.tensor_add(
                            t[:], ex2[:, 0:CHUNK], ex2[:, CHUNK : 2 * CHUNK])
                        lvl.append(t)
                        yield 1024
                while len(lvl) > 1:
                    nxt = []
                    for p in range(0, len(lvl) - 1, 2):
                        t = acc_pool.tile([P, CHUNK], BF16, name="acc")
                        nc.vector.tensor_add(t[:], lvl[p][:], lvl[p + 1][:])
                        nxt.append(t)
                    if len(lvl) % 2:
                        nxt.append(lvl[-1])
                    lvl = nxt
                den_ps = psum_o.tile([P, CHUNK], F32, name="ps_o")
                nc.tensor.matmul(den_ps[:], ones_t[:], lvl[0][:], start=True, stop=True)
                rc = recip_pool.tile([P, CHUNK], F32, name="rc")
                nc.vector.reciprocal_approx_fast(rc[:], den_ps[:])
                nc.vector.tensor_mul(outT[h][:, i0 : i0 + CHUNK], out_ps[:], rc[:])

        with tc.tile_pool(name="wq", bufs=1) as wq_pool, tc.tile_pool(
            name="xTs", bufs=1
        ) as xT_pool:
            # ---- streamed weight/xT loads ----
            wq_tiles = []
            xT_tiles = []
            dma_q = [nc.sync, nc.scalar, nc.gpsimd]
            crit_q = [
                nc.gpsimd if (eb % 2 == 1 and eb != 1) else (
                    nc.sync if eb % 4 == 0 else nc.scalar)
                for eb in range(EB)
            ]
            for eb in range(EB):
                q = crit_q[eb]
                xt = xT_pool.tile([P, S], BF16, name=f"xT{eb}")
                (nc.scalar if eb == 0 else q).dma_start(
                    xt[:, 0:CHUNK], xT_d[eb][:, 0:CHUNK]
                )
                xT_tiles.append(xt)
                t = wq_pool.tile([P, NJ * P], BF16, name=f"wq{eb}")
                if eb < 4:
                    q.dma_start(t[:, 0:P], wqkv_d[eb][:, 0:P])
                    q.dma_start(t[:, P : NQK * P], wqkv_d[eb][:, P : NQK * P])
                else:
                    q.dma_start(t[:, 0 : NQK * P], wqkv_d[eb][:, 0 : NQK * P])
                wq_tiles.append(t)
            for eb in range(EB):
                crit_q[eb].dma_start(
                    wq_tiles[eb][:, NQK * P : NJ * P], wqkv_d[eb][:, NQK * P : NJ * P]
                )
            nc.sync.dma_start(bq_t[:], bqkv[:])
            nc.scalar.dma_start(bv_t[:], bv[:])
            nc.sync.dma_start(tri_t[:], tri[:])
            nc.scalar.dma_start(ones_t[:], ones[:])
            qi = 0
            for sc in range(1, SC):
                s0 = sc * CHUNK
                for eb in range(EB):
                    dma_q[qi % 3].dma_start(
                        xT_tiles[eb][:, s0 : s0 + CHUNK], xT_d[eb][:, s0 : s0 + CHUNK]
                    )
                    qi += 1

            # ---- phase 1 chunk 0: eb-major across all 8 banks (paced with
            # the streaming loads) ----
            arrival = [0, 1, 3, 2, 5, 4, 6, 7, 9, 8, 10, 11, 13, 12, 14, 15]
            ps = [psum_k.tile([P, CHUNK], F32, name="ps_k") for _ in range(2)]
            ps += [psum_o.tile([P, CHUNK], F32, name="ps_o") for _ in range(2)]
            pA = psum_s2.tile([P, 2 * CHUNK], F32, name="ps_s2")
            pB = psum_s2.tile([P, 2 * CHUNK], F32, name="ps_s2")
            ps += [pA[:, 0:CHUNK], pA[:, CHUNK:], pB[:, 0:CHUNK], pB[:, CHUNK:]]
            for k, eb in enumerate(arrival):
                for jb in range(NQK):
                    nc.tensor.matmul(
                        ps[jb][:],
                        wq_tiles[eb][:, jb * P : (jb + 1) * P],
                        xT_tiles[eb][:, 0:CHUNK],
                        start=(k == 0),
                        stop=(k == EB - 1),
                    )
            for jb in range(NQK):
                nc.vector.tensor_scalar_add(
                    qkT[jb][:, 0:CHUNK], ps[jb][:], bq_t[:, jb : jb + 1]
                )
            for r in range(DIAG):
                pv = psum_k.tile([P, H * P], F32, name="ps_k")
                for eb in range(EB):
                    nc.tensor.matmul(
                        pv[:],
                        xT_tiles[eb][:, r * P : (r + 1) * P],
                        wq_tiles[eb][:, NQK * P : NJ * P],
                        start=(eb == 0),
                        stop=(eb == EB - 1),
                    )
                nc.vector.tensor_add(vnat[r][:], pv[:], bv_t[:])

            # ---- phase 1 chunks 1..3: jb-major, with chunk ci=sc-1's
            # attention units sliced into the stream ----
            for sc in range(1, SC):
                s0 = sc * CHUNK
                feeder = p2_chunk_feeder(sc - 1)
                per_slot = (16 * sc + 11) // 12
                for jb in range(NQK):
                    pk = psum_k.tile([P, CHUNK], F32, name="ps_k")
                    for eb in range(EB):
                        nc.tensor.matmul(
                            pk[:],
                            wq_tiles[eb][:, jb * P : (jb + 1) * P],
                            xT_tiles[eb][:, s0 : s0 + CHUNK],
                            start=(eb == 0),
                            stop=(eb == EB - 1),
                        )
                    nc.vector.tensor_scalar_add(
                        qkT[jb][:, s0 : s0 + CHUNK], pk[:], bq_t[:, jb : jb + 1]
                    )
                    for _ in range(per_slot):
                        next(feeder, None)
                for r in range(DIAG):
                    sb = sc * DIAG + r
                    pv = psum_k.tile([P, H * P], F32, name="ps_k")
                    for eb in range(EB):
                        nc.tensor.matmul(
                            pv[:],
                            xT_tiles[eb][:, s0 + r * P : s0 + (r + 1) * P],
                            wq_tiles[eb][:, NQK * P : NJ * P],
                            start=(eb == 0),
                            stop=(eb == EB - 1),
                        )
                    nc.vector.tensor_add(vnat[sb][:], pv[:], bv_t[:])
                    for _ in range(per_slot):
                        next(feeder, None)
                for _ in feeder:  # finish any remainder of the chunk's units
                    pass

        # ---- post-phase-1: wp loads, chunk-3 attention, all of c_proj ----
        wp_pool = ctx.enter_context(tc.tile_pool(name="wp", bufs=1))
        yout_pool = ctx.enter_context(tc.tile_pool(name="yout", bufs=2))
        wp_tiles = []
        for hb in range(H):
            t = wp_pool.tile([P, E], BF16, name=f"wp{hb}")
            dma_q[hb % 3].dma_start(t[:], wproj_d[hb])
            wp_tiles.append(t)

        def p3_group_gen(ci, tail=False):
            for r in range(DIAG):
                sb = ci * DIAG + r
                ot = yout_pool.tile([P, E], BF16, name="yo")
                for ec in range(EC):
                    ps3 = psum_k.tile([P, CHUNK], F32, name="ps_k")
                    for h in range(H):
                        nc.tensor.matmul(
                            ps3[:],
                            outT[h][:, sb * P : (sb + 1) * P],
                            wp_tiles[h][:, ec * CHUNK : (ec + 1) * CHUNK],
                            start=(h == 0),
                            stop=(h == H - 1),
                        )
                    if (tail and ec % 2 == 0) or (not tail and (sb + ec) % 4 == 0):
                        nc.scalar.copy(ot[:, ec * CHUNK : (ec + 1) * CHUNK], ps3[:])
                    else:
                        nc.vector.tensor_copy(
                            ot[:, ec * CHUNK : (ec + 1) * CHUNK], ps3[:]
                        )
                    if sb == SB - 1:
                        (nc.gpsimd if ec % 2 == 0 else nc.sync).dma_start(
                            y[sb * P : (sb + 1) * P, ec * CHUNK : (ec + 1) * CHUNK],
                            ot[:, ec * CHUNK : (ec + 1) * CHUNK],
                        )
                yield
                if sb < SB - 1:
                    [nc.sync, nc.gpsimd][sb % 2].dma_start(
                        y[sb * P : (sb + 1) * P, :], ot[:]
                    )

        # chunk-3 attention with c_proj groups (chunks 0-2) as deficit-paced
        # PE filler, then the remaining groups as the tail
        def p3_all():
            for ci in range(SC):
                yield from p3_group_gen(ci, tail=(ci == SC - 1))

        filler = p3_all()
        feeder = p2_chunk_feeder(SC - 1)
        deficit = 0.0
        for w in feeder:
            deficit += (0.15 + w * 0.00098) - w * 0.000834
            if deficit >= 2.6:  # one c_proj row (4 groups) per fill
                next(filler, None)
                deficit -= 3.44
        for _ in filler:
            pass
    return nc


_NC = None
LAST_RESULTS = None


def _get_nc():
    global _NC
    if _NC is None:
        nc = bacc.Bacc(
            "TRN2", target_bir_lowering=False, debug=False, num_devices=N_CORES
        )
        _emit(nc)
        nc.compile()
        _NC = nc
    return _NC


def _prep_shared(hidden_states, c_attn_w, c_attn_b, c_proj_w):
    import ml_dtypes

    bf16 = ml_dtypes.bfloat16
    scale = 1.0 / float(np.sqrt(P))
    xT = [
        np.ascontiguousarray(hidden_states[b].T).astype(bf16) for b in range(BATCH)
    ]
    pp = np.arange(P)
    tri = (pp[:, None] <= pp[None, :]).astype(bf16)
    ones = np.ones((P, P), dtype=bf16)
    return xT, tri, ones, scale, bf16


def _core_inputs(shared, c_attn_w, c_attn_b, c_proj_w, core):
    xT, tri, ones, scale, bf16 = shared
    b, g = core // 4, core % 4
    h0 = H * g
    cols = []
    for part in range(3):
        for h in range(h0, h0 + H):
            base = part * E + h * P
            cols.extend(range(base, base + P))
    cols = np.asarray(cols)
    wqkv = np.ascontiguousarray(c_attn_w[:, cols]).astype(np.float32)
    bq = np.ascontiguousarray(c_attn_b[cols]).astype(np.float32)
    wqkv[:, 0 : H * P] *= scale
    bq[0 : H * P] *= scale
    bq = bq.reshape(NJ, P).T.copy()
    wproj = np.ascontiguousarray(c_proj_w[h0 * P : (h0 + H) * P, :]).astype(bf16)
    bv = np.broadcast_to(bq[:, NQK:NJ].T.reshape(1, H * P), (P, H * P)).copy()
    return {
        "xT": xT[b],
        "wqkv": wqkv.astype(bf16),
        "bqkv": bq,
        "wproj": wproj,
        "tri": tri,
        "ones": ones,
        "bv": bv,
    }


def kernel(hidden_states, c_attn_w, c_attn_b, c_proj_w, c_proj_b):
    global LAST_RESULTS
    hidden_states = np.asarray(hidden_states)
    c_attn_w = np.asarray(c_attn_w)
    c_attn_b = np.asarray(c_attn_b)
    c_proj_w = np.asarray(c_proj_w)
    c_proj_b = np.asarray(c_proj_b)

    nc = _get_nc()
    shared = _prep_shared(hidden_states, c_attn_w, c_attn_b, c_proj_w)
    in_maps = [
        _core_inputs(shared, c_attn_w, c_attn_b, c_proj_w, c)
        for c in range(N_CORES)
    ]
    res = run_bass_kernel_spmd(nc, in_maps, list(range(N_CORES)))
    LAST_RESULTS = res
    out = np.zeros((BATCH, S, E), dtype=np.float32)
    for c in range(N_CORES):
        out[c // 4] += res.results[c]["y"].astype(np.float32)
    out += c_proj_b.astype(np.float32)[None, None, :]
    return out
